# revision 22
# baseline (speedup 1.0000x reference)
"""Fused GPT transformer layer on 8 trn2 cores — token-parallel + KV AllGather.

Sharding: core i owns 512 contiguous tokens (cores 0-3 batch 0, 4-7 batch 1).
Per core: LN1 -> QKV (+RoPE) local; AllGather K^T,V within 4-core group;
masked full-key attention (softmax without max-subtraction — safe since
scores ~ N(0,1) for LN'd inputs); dense+residual, LN2, fused chunked MLP all
local. Host gathers per-core outputs.

v3: all matmul operands bf16 (FWL weight loads, half DMA/SBUF), fp32 PSUM
accumulation and fp32 LN/residual/softmax-normalization. Order K -> AG(K) ->
Q -> V -> AG(V) -> attention so local compute hides both collectives.

Layouts:  "N" = [token-partition, feature-free]; "T" = [feature-part, token-free].
"""
import sys
if '/opt/trn_rl_repo' not in sys.path:
    sys.path.insert(0, '/opt/trn_rl_repo')

from dataclasses import dataclass

import numpy as np
import ml_dtypes

import concourse.bass as bass
import concourse.bacc as bacc
import concourse.tile as tile
import concourse.mybir as mybir
from concourse import bass_utils
from concourse.masks import make_identity
from concourse.replica_groups import maybe_share_collective_output_space

F32 = mybir.dt.float32
BF16 = mybir.dt.bfloat16
U8 = mybir.dt.uint8
AF = mybir.ActivationFunctionType
ALU = mybir.AluOpType
BF = ml_dtypes.bfloat16


@dataclass
class Cfg:
    B: int = 2
    S: int = 2048
    H: int = 2048
    NH: int = 16
    FF: int = 8192
    W: int = 8           # total cores
    FC: int = 1024       # FF chunk for fused MLP
    WSP: int = 512       # weight panel span (moving free dim for N-layout mms)
    EPS: float = 1e-5
    sim_gelu: bool = False
    phase_limit: int = 99   # 1=A, 2=B(+AG), 3=C, 4=D, 5=E

    @property
    def HD(self):
        return self.H // self.NH

    @property
    def NG(self):
        return self.W // self.B

    @property
    def T(self):
        return self.S // self.NG

    @property
    def NT(self):
        return self.T // 128

    @property
    def KH(self):
        return self.H // 128

    @property
    def NOS(self):
        return self.H // self.WSP

    @property
    def SCALE(self):
        return 1.0 / float(np.sqrt(self.HD))


def build(cfg: Cfg):
    c = cfg
    assert c.HD == 128 and c.T % 128 == 0 and c.H % c.WSP == 0
    assert c.FF % c.FC == 0 and c.FC % 128 == 0

    nc = bacc.Bacc("TRN2", target_bir_lowering=False, debug=False,
                   num_devices=c.W)
    d = lambda name, shape, dt=F32: nc.dram_tensor(name, shape, dt,
                                                   kind="ExternalInput")
    io = {}
    io["x_in"] = d("x", [c.T, c.H])
    # wq/wk host-relaid as [128, NH*KH*128]: col block h*KH*128 holds the
    # head-h panel in (kk, f) order -> contiguous single-descriptor DMA.
    io["wq_in"] = d("wq", [128, c.NH * c.KH * 128], BF16)
    io["wk_in"] = d("wk", [128, c.NH * c.KH * 128], BF16)
    io["wv_in"] = d("wv", [c.H, c.H], BF16)
    io["wd_in"] = d("wd", [c.H, c.H], BF16)
    io["w1_in"] = d("w1", [c.H, c.FF], BF16)
    io["w2_in"] = d("w2", [c.FF, c.H], BF16)
    io["bq_in"] = d("bq", [1, c.H], BF16)
    io["bk_in"] = d("bk", [1, c.H], BF16)
    io["bd_in"] = d("bd", [1, c.H], BF16)   # holds bv@wd + b_dense
    io["b1_in"] = d("b1", [128, c.FF // 128])
    io["b2_in"] = d("b2", [1, c.H], BF16)
    io["ones_r_in"] = d("ones_r", [1, c.T], BF16)
    io["ones_c_in"] = d("ones_c", [128, 1], BF16)
    io["cos_in"] = d("cosT", [128, c.T])
    io["sins_in"] = d("sinsT", [128, c.T])
    io["mask_in"] = d("maskT", [c.S, c.T], BF16)
    io["out_ext"] = nc.dram_tensor("out", [c.T, c.H], F32, kind="ExternalOutput")
    io["groups"] = [[g * c.NG + r for r in range(c.NG)] for g in range(c.B)]

    with tile.TileContext(nc) as tc:
        _body(nc, tc, c, io)
    nc.compile()
    return nc


def _body(nc, tc, c, io):
    x_in, out_ext = io["x_in"], io["out_ext"]
    NT, KH, NH, T, H = c.NT, c.KH, c.NH, c.T, c.H
    WSP, NOS = c.WSP, c.NOS
    SKT = c.S // 128
    AXX = mybir.AxisListType.X

    # ---------- persistent pools ----------
    const = tc.alloc_tile_pool(name="const", bufs=1)
    ident = const.tile([128, 128], F32, tag="ident", name="ident")
    make_identity(nc, ident[:])
    ones_r = const.tile([1, T], BF16, tag="ones_r", name="ones_r")
    nc.sync.dma_start(ones_r[:], io["ones_r_in"].ap()[:])
    ones_c = const.tile([128, 1], BF16, tag="ones_c", name="ones_c")
    nc.sync.dma_start(ones_c[:], io["ones_c_in"].ap()[:])
    epsap = const.tile([128, 1], F32, tag="epsap", name="epsap")
    nc.gpsimd.memset(epsap[:], c.EPS)
    b1_sb = const.tile([128, c.FF // 128], F32, tag="b1", name="b1")
    nc.sync.dma_start(b1_sb[:], io["b1_in"].ap()[:])
    cos_sb = const.tile([128, T], F32, tag="cos", name="cos")
    nc.sync.dma_start(cos_sb[:], io["cos_in"].ap()[:])
    sins_sb = const.tile([128, T], F32, tag="sins", name="sins")
    nc.sync.dma_start(sins_sb[:], io["sins_in"].ap()[:])

    stat = tc.alloc_tile_pool(name="stat", bufs=2)
    big = tc.alloc_tile_pool(name="big", bufs=1)
    pp = tc.alloc_tile_pool(name="pp", bufs=1, space="PSUM")
    dram = tc.alloc_tile_pool(name="dram", bufs=1, space="DRAM")

    HC = NH // 2        # heads per K AllGather chunk
    ag_space = maybe_share_collective_output_space("AllGather", io["groups"])
    kT_bnc = [dram.tile([HC * 128, T], BF16, tag="kTb", name=f"kTb{ci}",
                        bufs=2) for ci in range(2)]
    kT_allc = [dram.tile([c.NG * HC * 128, T], BF16, tag="kTall",
                         name=f"kTall{ci}", bufs=2, addr_space=ag_space)
               for ci in range(2)]
    v_bnc = [dram.tile([256, H], BF16, tag="vb", name=f"vb{ci}", bufs=2)
             for ci in range(2)]
    v_alls = [dram.tile([c.NG * 256, H], BF16, tag="vall", name=f"vall{ci}",
                        bufs=2, addr_space=ag_space) for ci in range(2)]

    def ln_tile(src, out, scrpool):
        """LN stats + normalize for one N-layout tile [128, H] -> out."""
        s1 = stat.tile([128, 1], F32, tag="s1", name="s1")
        nc.vector.tensor_reduce(s1[:], src[:], axis=AXX, op=ALU.add)
        scr = scrpool.tile([128, H], F32, tag="lnscr", name="lnscr", bufs=2)
        nc.scalar.square(scr[:], src[:])
        s2 = stat.tile([128, 1], F32, tag="s2", name="s2")
        nc.vector.tensor_reduce(s2[:], scr[:], axis=AXX, op=ALU.add)
        m = stat.tile([128, 1], F32, tag="m", name="m")
        nc.scalar.mul(m[:], s1[:], 1.0 / H)
        msq = stat.tile([128, 1], F32, tag="msq", name="msq")
        nc.vector.scalar_tensor_tensor(msq[:], m[:], 1.0, m[:],
                                       op0=ALU.mult, op1=ALU.mult)
        var = stat.tile([128, 1], F32, tag="var", name="var")
        nc.vector.scalar_tensor_tensor(var[:], s2[:], 1.0 / H, msq[:],
                                       op0=ALU.mult, op1=ALU.subtract)
        std = stat.tile([128, 1], F32, tag="std", name="std")
        nc.scalar.activation(std[:], var[:], AF.Sqrt, bias=epsap[:], scale=1.0)
        rstd = stat.tile([128, 1], F32, tag="rstd", name="rstd")
        nc.vector.reciprocal(rstd[:], std[:])
        negmr = stat.tile([128, 1], F32, tag="negmr", name="negmr")
        nc.vector.scalar_tensor_tensor(negmr[:], m[:], -1.0, rstd[:],
                                       op0=ALU.mult, op1=ALU.mult)
        nc.scalar.activation(out[:], src[:], AF.Identity,
                             bias=negmr[:], scale=rstd[:])

    def transpose_tile(srcN, dstT_list, t):
        """[128tok, H] f32 -> cast into KH bf16 T-layout tiles at column t."""
        for kk in range(KH):
            ps = pp.tile([128, 128], F32, tag="ptr", name="ptr", bufs=1)
            nc.tensor.transpose(ps[:], srcN[:, 128 * kk:128 * (kk + 1)], ident[:])
            nc.vector.tensor_copy(dstT_list[kk][:, 128 * t:128 * (t + 1)], ps[:])

    # ---------- Phase A: LN1 + transpose (x streamed) ----------
    pa = tc.alloc_tile_pool(name="pa", bufs=1)
    xT = [big.tile([128, T], BF16, tag="TT", name=f"xT{kk}", bufs=KH)
          for kk in range(KH)]
    for t in range(NT):
        xt = pa.tile([128, H], F32, tag="ax", name=f"x{t}", bufs=3)
        nc.sync.dma_start(xt[:], x_in.ap()[128 * t:128 * (t + 1), :])
        xh = pa.tile([128, H], F32, tag="axh", name=f"xh{t}", bufs=3)
        ln_tile(xt, xh, pa)
        transpose_tile(xh, xT, t)
    pa.release()

    # ---------- Phase B: K (2 chunked AGs) -> V (tiled AGs) -> Q ----------
    pcd = tc.alloc_tile_pool(name="pcd", bufs=1)   # ctx tiles (live into D)
    pb = tc.alloc_tile_pool(name="pb", bufs=1)
    qT = [big.tile([128, T], BF16, tag="qT", name=f"qT{h}", bufs=NH)
          for h in range(NH)]

    def qk_head(h, w_in, b_in, dst):
        """dst: (dram_ap, row0) or sbuf tile"""
        ps = pp.tile([128, T], F32, tag="mm", name="pqk", bufs=2)
        wt = pb.tile([128, KH * 128], BF16, tag="wqk", name="wqk", bufs=4)
        nc.sync.dma_start(
            wt[:], w_in.ap()[:, h * KH * 128:(h + 1) * KH * 128])
        for kk in range(KH):
            nc.tensor.matmul(ps[:], wt[:, 128 * kk:128 * (kk + 1)], xT[kk][:],
                             start=(kk == 0), stop=False)
        bt = pb.tile([1, 128], BF16, tag="bqk", name="bqk", bufs=2)
        nc.sync.dma_start(bt[:], b_in.ap()[:, 128 * h:128 * (h + 1)])
        nc.tensor.matmul(ps[:], bt[:], ones_r[:], start=False, stop=True)
        tmp = pb.tile([128, T], F32, tag="ropetmp", name="ropetmp", bufs=3)
        nc.vector.scalar_tensor_tensor(tmp[0:64, :], ps[64:128, :], 1.0,
                                       sins_sb[0:64, :], op0=ALU.mult,
                                       op1=ALU.mult)
        nc.vector.scalar_tensor_tensor(tmp[64:128, :], ps[0:64, :], 1.0,
                                       sins_sb[64:128, :], op0=ALU.mult,
                                       op1=ALU.mult)
        qc = pb.tile([128, T], F32, tag="ropeqc", name="ropeqc", bufs=3)
        nc.vector.scalar_tensor_tensor(qc[:], ps[:], 1.0, cos_sb[:],
                                       op0=ALU.mult, op1=ALU.mult)
        if isinstance(dst, tuple):
            res = pb.tile([128, T], BF16, tag="qkres", name="qkres", bufs=3)
            nc.vector.scalar_tensor_tensor(res[:], qc[:], 1.0, tmp[:],
                                           op0=ALU.mult, op1=ALU.add)
            d_ap, row0 = dst
            nc.sync.dma_start(d_ap[row0:row0 + 128, :], res[:])
        else:
            nc.vector.scalar_tensor_tensor(dst[:], qc[:], 1.0, tmp[:],
                                           op0=ALU.mult, op1=ALU.add)

    # K first, AllGather per 8-head chunk so the CC stream starts early.
    for ci in range(2):
        for hl in range(HC):
            qk_head(ci * HC + hl, io["wk_in"], io["bk_in"],
                    (kT_bnc[ci], 128 * hl))
        nc.gpsimd.collective_compute(
            "AllGather", ALU.bypass, ins=[kT_bnc[ci].opt()],
            outs=[kT_allc[ci].opt()], replica_groups=io["groups"])

    # V next so its AGs queue right behind K's chunks (2 chunks of 2 tiles).
    ppv = tc.alloc_tile_pool(name="ppv", bufs=1, space="PSUM")
    pv = tc.alloc_tile_pool(name="pv", bufs=1)
    wv_sb = []
    for kk in range(KH):
        wt = pv.tile([128, H], BF16, tag="wvpan", name="wvpan", bufs=KH)
        nc.sync.dma_start(wt[:], io["wv_in"].ap()[128 * kk:128 * (kk + 1), :])
        wv_sb.append(wt)
    for ci in range(2):
        for tl in range(2):
            t = 2 * ci + tl
            pss = [ppv.tile([128, WSP], F32, tag=f"pvac{o}", name=f"pvac{o}",
                            bufs=1) for o in range(NOS)]
            for kk in range(KH):
                lhs = xT[kk][:, 128 * t:128 * (t + 1)]
                for osp in range(NOS):
                    nc.tensor.matmul(pss[osp][:], lhs,
                                     wv_sb[kk][:, WSP * osp:WSP * (osp + 1)],
                                     start=(kk == 0), stop=(kk == KH - 1))
            for osp in range(NOS):
                vs = pb.tile([128, WSP], BF16, tag="vslice", name="vslice",
                             bufs=3)
                nc.vector.tensor_copy(vs[:], pss[osp][:])
                nc.sync.dma_start(
                    v_bnc[ci][128 * tl:128 * (tl + 1),
                              WSP * osp:WSP * (osp + 1)], vs[:])
        nc.gpsimd.collective_compute(
            "AllGather", ALU.bypass, ins=[v_bnc[ci].opt()],
            outs=[v_alls[ci].opt()], replica_groups=io["groups"])
    pv.release()
    ppv.release()

    # ---------- Phase C: attention, k-token-tile-major (AG-arrival order) ---
    pc = tc.alloc_tile_pool(name="pc", bufs=1)
    ppc = tc.alloc_tile_pool(name="ppc", bufs=1, space="PSUM")
    mask_sb = [pc.tile([128, T], BF16, tag="mask", name=f"mask{m}", bufs=SKT)
               for m in range(SKT)]
    for m in range(SKT):
        nc.sync.dma_start(mask_sb[m][:],
                          io["mask_in"].ap()[128 * m:128 * (m + 1), :])
    ctxacc = [pc.tile([128, T], F32, tag="ctxacc", name=f"ca{h}", bufs=NH)
              for h in range(NH)]
    # softmax denominators packed 4-per-tile at 32-aligned partition bases
    sum_tiles = [pc.tile([128, T], BF16, tag="sumacc", name=f"sumacc{i}",
                         bufs=4) for i in range(4)]
    sslice = lambda h: sum_tiles[h // 4][32 * (h % 4):32 * (h % 4) + 1, :]
    ctx = [None] * NH
    for lm in range(NT):
        for h in range(NH):
            if lm == 0:
                # JIT Q: compute head h+1's Q while head h's rope drains,
                # so the score matmuls never wait on the vector engine.
                if h == 0:
                    qk_head(0, io["wq_in"], io["bq_in"], qT[0])
                if h + 1 < NH:
                    qk_head(h + 1, io["wq_in"], io["bq_in"], qT[h + 1])
            ci, hl = divmod(h, HC)
            # K block panel for (lm, h): [128 hd, r-major 4x128 keys].
            kp = pc.tile([128, c.NG * 128], BF16, tag="kpan", name="kpan",
                         bufs=6)
            nc.gpsimd.dma_start(
                kp[:].rearrange("p (r f) -> p r f", f=128),
                kT_allc[ci].rearrange("(r hh p) t -> p hh r t", hh=HC, p=128)
                [:, hl, :, 128 * lm:128 * (lm + 1)])
            vp = pc.tile([128, c.NG * 128], BF16, tag="vpan", name="vpan",
                         bufs=6)
            nc.sync.dma_start(
                vp[:].rearrange("p (r f) -> p r f", f=128),
                v_alls[lm // 2].rearrange("(r tt p) cc -> p tt r cc",
                                          tt=2, p=128)
                [:, lm % 2, :, 128 * h:128 * (h + 1)])
            ems = []
            for r in range(c.NG):
                m = r * NT + lm
                ps_s = ppc.tile([128, T], F32, tag="pscore", name="pscore",
                                bufs=2)
                nc.tensor.matmul(ps_s[:], kp[:, 128 * r:128 * (r + 1)],
                                 qT[h][:], start=True, stop=True)
                e_m = pc.tile([128, T], BF16, tag="eatt", name="eatt", bufs=3)
                nc.scalar.activation(e_m[:], ps_s[:], AF.Exp, bias=0.0,
                                     scale=c.SCALE)
                em2 = pc.tile([128, T], BF16, tag="eatt2", name="eatt2",
                              bufs=5)
                nc.vector.scalar_tensor_tensor(em2[:], e_m[:], 1.0,
                                               mask_sb[m][:], op0=ALU.mult,
                                               op1=ALU.mult)
                ems.append(em2)
            ps_ctx = ppc.tile([128, T], F32, tag="pctx", name="pctx", bufs=2)
            for r in range(c.NG):
                nc.tensor.matmul(ps_ctx[:], vp[:, 128 * r:128 * (r + 1)],
                                 ems[r][:], start=(r == 0),
                                 stop=(r == c.NG - 1))
            ps_sum = ppc.tile([1, T], F32, tag="psml", name="psml", bufs=1)
            for r in range(c.NG):
                nc.tensor.matmul(ps_sum[:], ones_c[:], ems[r][:],
                                 start=(r == 0), stop=(r == c.NG - 1))
            if lm == 0:
                nc.vector.tensor_copy(ctxacc[h][:], ps_ctx[:])
                nc.vector.tensor_copy(sslice(h), ps_sum[:])
            else:
                nc.vector.scalar_tensor_tensor(ctxacc[h][:], ps_ctx[:], 1.0,
                                               ctxacc[h][:], op0=ALU.mult,
                                               op1=ALU.add)
                nc.vector.scalar_tensor_tensor(sslice(h), ps_sum[:],
                                               1.0, sslice(h),
                                               op0=ALU.mult, op1=ALU.add)
            if lm == NT - 1:
                # normalize head h right away — overlaps later heads' scores
                rsum = stat.tile([1, T], F32, tag="rsum", name="rsum")
                nc.vector.reciprocal(rsum[:], sslice(h))
                rrep = stat.tile([128, T], F32, tag="rsumrep", name="rsumrep")
                nc.gpsimd.partition_broadcast(rrep[:], rsum[:])
                cn = pcd.tile([128, T], BF16, tag="ctx", name=f"ctx{h}",
                              bufs=NH)
                nc.vector.scalar_tensor_tensor(cn[:], ctxacc[h][:], 1.0,
                                               rrep[:], op0=ALU.mult,
                                               op1=ALU.mult)
                ctx[h] = cn
    ppc.release()
    pc.release()
    pb.release()

    # ---------- Phase D: dense + residual, LN2, transpose ----------
    pd = tc.alloc_tile_pool(name="pd", bufs=1)
    hid_b = dram.tile([T, H], F32, tag="hidb", name="hidb")
    ppd = tc.alloc_tile_pool(name="ppd", bufs=1, space="PSUM")
    wd_sb = []
    for kk in range(KH):
        wt = pd.tile([128, H], BF16, tag="wdpan", name="wdpan", bufs=KH)
        nc.sync.dma_start(wt[:], io["wd_in"].ap()[128 * kk:128 * (kk + 1), :])
        wd_sb.append(wt)
    bts = []
    for osp in range(NOS):
        bt = pd.tile([1, WSP], BF16, tag="bdsl", name="bdsl", bufs=NOS)
        nc.sync.dma_start(bt[:], io["bd_in"].ap()[:, WSP * osp:WSP * (osp + 1)])
        bts.append(bt)
    for t in range(NT):
        pss = [ppd.tile([128, WSP], F32, tag=f"pdac{o}", name=f"pdac{o}",
                        bufs=1) for o in range(NOS)]
        for kk in range(KH):
            lhs = ctx[kk][:, 128 * t:128 * (t + 1)]
            for osp in range(NOS):
                nc.tensor.matmul(pss[osp][:], lhs,
                                 wd_sb[kk][:, WSP * osp:WSP * (osp + 1)],
                                 start=(kk == 0), stop=False)
        for osp in range(NOS):
            nc.tensor.matmul(pss[osp][:], ones_r[:, 0:128], bts[osp][:],
                             start=False, stop=True)
            xs = pd.tile([128, WSP], F32, tag="xsl", name="xsl", bufs=3)
            nc.sync.dma_start(
                xs[:], x_in.ap()[128 * t:128 * (t + 1),
                                 WSP * osp:WSP * (osp + 1)])
            hs = pd.tile([128, WSP], F32, tag="hsl", name="hsl", bufs=3)
            nc.vector.scalar_tensor_tensor(hs[:], pss[osp][:], 1.0, xs[:],
                                           op0=ALU.mult, op1=ALU.add)
            nc.sync.dma_start(
                hid_b[128 * t:128 * (t + 1), WSP * osp:WSP * (osp + 1)], hs[:])
    ppd.release()
    pd.release()
    pcd.release()

    pdh = tc.alloc_tile_pool(name="pdh", bufs=1)
    hT = [big.tile([128, T], BF16, tag="TT", name=f"hT{kk}", bufs=KH)
          for kk in range(KH)]
    for t in range(NT):
        ht = pdh.tile([128, H], F32, tag="dh", name=f"hid{t}", bufs=3)
        nc.sync.dma_start(ht[:], hid_b[128 * t:128 * (t + 1), :])
        hh = pdh.tile([128, H], F32, tag="dhh", name=f"hh{t}", bufs=3)
        ln_tile(ht, hh, pdh)
        transpose_tile(hh, hT, t)
    pdh.release()

    # ---------- Phase E: fused MLP ----------
    pe = tc.alloc_tile_pool(name="pe", bufs=1)
    ppe2 = tc.alloc_tile_pool(name="ppe2", bufs=1, space="PSUM")
    NFC = c.FF // c.FC
    FCT = c.FC // 128
    out_t = [big.tile([128, H], F32, tag="bigH", name=f"out{t}", bufs=4)
             for t in range(NT)]
    for f in range(NFC):
        gT = []
        for mm in range(FCT):
            fglob = f * FCT + mm
            w1t = pe.tile([128, KH * 128], BF16, tag="w1pan", name="w1pan",
                          bufs=4)
            nc.sync.dma_start(
                w1t[:].rearrange("p (kk f) -> p kk f", f=128),
                io["w1_in"].ap()[:, 128 * fglob:128 * (fglob + 1)]
                .rearrange("(kk p) f -> p kk f", p=128))
            ps = pp.tile([128, T], F32, tag="mm", name="pm1", bufs=2)
            for kk in range(KH):
                nc.tensor.matmul(ps[:], w1t[:, 128 * kk:128 * (kk + 1)],
                                 hT[kk][:], start=(kk == 0),
                                 stop=(kk == KH - 1))
            g = pe.tile([128, T], BF16, tag="gT", name="gT", bufs=FCT + 4)
            if c.sim_gelu:
                a = pe.tile([128, T], F32, tag="ga", name="ga", bufs=2)
                nc.scalar.activation(a[:], ps[:], AF.Identity,
                                     bias=b1_sb[:, fglob:fglob + 1], scale=1.0)
                sg = pe.tile([128, T], F32, tag="gsg", name="gsg", bufs=2)
                nc.scalar.activation(sg[:], a[:], AF.Sigmoid, bias=0.0,
                                     scale=1.702)
                nc.vector.scalar_tensor_tensor(g[:], a[:], 1.0, sg[:],
                                               op0=ALU.mult, op1=ALU.mult)
            else:
                nc.scalar.activation(g[:], ps[:], AF.Gelu,
                                     bias=b1_sb[:, fglob:fglob + 1], scale=1.0)
            gT.append(g)
        w2_sb = []
        for kf in range(FCT):
            wt = pe.tile([128, H], BF16, tag="w2pan", name="w2pan",
                         bufs=FCT + 4)
            nc.sync.dma_start(
                wt[:], io["w2_in"].ap()[128 * (f * FCT + kf):
                                        128 * (f * FCT + kf + 1), :])
            w2_sb.append(wt)
        if f == 0:
            b2s = []
            for osp in range(NOS):
                bt = pe.tile([1, WSP], BF16, tag="b2sl", name="b2sl", bufs=NOS)
                nc.sync.dma_start(
                    bt[:], io["b2_in"].ap()[:, WSP * osp:WSP * (osp + 1)])
                b2s.append(bt)
        for t in range(NT):
            pss = [ppe2.tile([128, WSP], F32, tag=f"pmac{o}", name=f"pmac{o}",
                             bufs=1) for o in range(NOS)]
            for kf in range(FCT):
                lhs = gT[kf][:, 128 * t:128 * (t + 1)]
                for osp in range(NOS):
                    nc.tensor.matmul(pss[osp][:], lhs,
                                     w2_sb[kf][:, WSP * osp:WSP * (osp + 1)],
                                     start=(kf == 0),
                                     stop=(kf == FCT - 1 and f != 0))
            for osp in range(NOS):
                osl = out_t[t][:, WSP * osp:WSP * (osp + 1)]
                if f == 0:
                    nc.tensor.matmul(pss[osp][:], ones_r[:, 0:128], b2s[osp][:],
                                     start=False, stop=True)
                    hsl = pe.tile([128, WSP], F32, tag="hres", name="hres",
                                  bufs=3)
                    nc.sync.dma_start(
                        hsl[:], hid_b[128 * t:128 * (t + 1),
                                      WSP * osp:WSP * (osp + 1)])
                    nc.vector.scalar_tensor_tensor(osl, pss[osp][:], 1.0,
                                                   hsl[:], op0=ALU.mult,
                                                   op1=ALU.add)
                else:
                    nc.vector.scalar_tensor_tensor(osl, pss[osp][:], 1.0, osl,
                                                   op0=ALU.mult, op1=ALU.add)
    ppe2.release()
    pe.release()

    # ---------- Phase F: output ----------
    for t in range(NT):
        nc.sync.dma_start(out_ext.ap()[128 * t:128 * (t + 1), :], out_t[t][:])

    for p in (pp, dram, big, stat, const):
        p.release()


# ---------------- host side ----------------

def prepare_in_maps(c: Cfg, inputs):
    f32 = np.float32
    hs = np.asarray(inputs["hidden_states"], f32)
    ln1_g = np.asarray(inputs["ln1_g"], f32)
    ln1_b = np.asarray(inputs["ln1_b"], f32)
    w_qkv = np.asarray(inputs["w_qkv"], f32)
    b_qkv = np.asarray(inputs["b_qkv"], f32)
    w_dense = np.asarray(inputs["w_dense"], f32)
    b_dense = np.asarray(inputs["b_dense"], f32)
    ln2_g = np.asarray(inputs["ln2_g"], f32)
    ln2_b = np.asarray(inputs["ln2_b"], f32)
    w1 = np.asarray(inputs["w1"], f32)
    b1 = np.asarray(inputs["b1"], f32)
    w2 = np.asarray(inputs["w2"], f32)
    b2 = np.asarray(inputs["b2"], f32)

    H, NH, HD, FF = c.H, c.NH, c.HD, c.FF
    cols = np.concatenate([np.arange(h * 3 * HD, h * 3 * HD + HD)
                           for h in range(NH)])
    wg = ln1_g[:, None] * w_qkv
    wq_f, wk_f, wv_f = wg[:, cols], wg[:, cols + HD], wg[:, cols + 2 * HD]
    bfull = ln1_b @ w_qkv + b_qkv
    bq_f, bk_f, bv_f = bfull[cols], bfull[cols + HD], bfull[cols + 2 * HD]
    bd_f = bv_f @ w_dense + b_dense          # v-bias folded through attention
    w1_f = ln2_g[:, None] * w1
    b1_f = ln2_b @ w1 + b1

    inv = 1.0 / (10000.0 ** (np.arange(0, HD, 2, dtype=f32) / HD))
    pos = np.arange(c.S, dtype=f32)
    frq = np.einsum('i,j->ij', pos, inv)
    emb = np.concatenate([frq, frq], axis=-1)
    cos_full = np.cos(emb).T.astype(f32)
    sin_full = np.sin(emb).T.astype(f32)
    sins_full = sin_full.copy()
    sins_full[:HD // 2] *= -1.0

    bf = lambda a: np.ascontiguousarray(a.astype(BF))
    KH = H // 128
    # [H, H] head-major -> [128, NH*KH*128]: col block h*KH*128+(kk*128+f)
    # = w[kk*128+p, h*128+f] (contiguous per-head panel for one-descriptor DMA)
    relay = lambda w: (w.reshape(KH, 128, NH, 128).transpose(1, 2, 0, 3)
                       .reshape(128, NH * KH * 128))
    wqT, wkT = relay(wq_f), relay(wk_f)
    in_maps = []
    for i in range(c.W):
        b, g = i // c.NG, i % c.NG
        t0 = g * c.T
        qpos = np.arange(t0, t0 + c.T)
        kpos = np.arange(c.S)
        mask = (kpos[:, None] <= qpos[None, :]).astype(BF)
        in_maps.append({
            "x": np.ascontiguousarray(hs[b, t0:t0 + c.T, :]),
            "wq": bf(wqT), "wk": bf(wkT), "wv": bf(wv_f),
            "wd": bf(w_dense), "w1": bf(w1_f), "w2": bf(w2),
            "bq": bf(bq_f.reshape(1, H)), "bk": bf(bk_f.reshape(1, H)),
            "bd": bf(bd_f.reshape(1, H)),
            "b1": np.ascontiguousarray(b1_f.reshape(FF // 128, 128).T),
            "b2": bf(b2.reshape(1, H)),
            "ones_r": np.ones((1, c.T), BF),
            "ones_c": np.ones((128, 1), BF),
            "cosT": np.ascontiguousarray(cos_full[:, t0:t0 + c.T]),
            "sinsT": np.ascontiguousarray(sins_full[:, t0:t0 + c.T]),
            "maskT": np.ascontiguousarray(mask),
        })
    return in_maps


def assemble_output(c: Cfg, results):
    out = np.empty((c.B, c.S, c.H), np.float32)
    for i in range(c.W):
        b, g = i // c.NG, i % c.NG
        out[b, g * c.T:(g + 1) * c.T, :] = results[i]["out"]
    return out


def run(nc, c: Cfg, inputs, trace=False, **kw):
    in_maps = prepare_in_maps(c, inputs)
    last = None
    for attempt in range(3):
        try:
            res = bass_utils.run_bass_kernel_spmd(
                nc, in_maps, core_ids=list(range(c.W)), trace=trace, **kw)
            return assemble_output(c, res.results), res
        except Exception as e:
            last = e
            print(f"run attempt {attempt} failed: {type(e).__name__}: {e}",
                  file=sys.stderr)
    raise last


# ======================= harness entry point =======================

_CACHE = {}


def kernel(**inputs):
    """Full-input entry: shard, compile (cached), run on 8 cores, gather."""
    c = Cfg()
    if "nc" not in _CACHE:
        _CACHE["nc"] = build(c)
    out, _ = run(_CACHE["nc"], c, inputs, trace=False)
    return out



# revision 26
# speedup vs baseline: 1.0154x; 1.0154x over previous
"""Fused GPT transformer layer on 8 trn2 cores — token-parallel + KV AllGather.

Sharding: core i owns 512 contiguous tokens (cores 0-3 batch 0, 4-7 batch 1).
Per core: LN1 -> QKV (+RoPE) local; AllGather K^T,V within 4-core group;
masked full-key attention (softmax without max-subtraction — safe since
scores ~ N(0,1) for LN'd inputs); dense+residual, LN2, fused chunked MLP all
local. Host gathers per-core outputs.

v3: all matmul operands bf16 (FWL weight loads, half DMA/SBUF), fp32 PSUM
accumulation and fp32 LN/residual/softmax-normalization. Order K -> AG(K) ->
Q -> V -> AG(V) -> attention so local compute hides both collectives.

Layouts:  "N" = [token-partition, feature-free]; "T" = [feature-part, token-free].
"""
import sys
if '/opt/trn_rl_repo' not in sys.path:
    sys.path.insert(0, '/opt/trn_rl_repo')

from dataclasses import dataclass

import numpy as np
import ml_dtypes

import concourse.bass as bass
import concourse.bacc as bacc
import concourse.tile as tile
import concourse.mybir as mybir
from concourse import bass_utils
from concourse.masks import make_identity
from concourse.replica_groups import maybe_share_collective_output_space

F32 = mybir.dt.float32
BF16 = mybir.dt.bfloat16
U8 = mybir.dt.uint8
AF = mybir.ActivationFunctionType
ALU = mybir.AluOpType
BF = ml_dtypes.bfloat16


@dataclass
class Cfg:
    B: int = 2
    S: int = 2048
    H: int = 2048
    NH: int = 16
    FF: int = 8192
    W: int = 8           # total cores
    FC: int = 1024       # FF chunk for fused MLP
    WSP: int = 512       # weight panel span (moving free dim for N-layout mms)
    EPS: float = 1e-5
    sim_gelu: bool = False
    phase_limit: int = 99   # 1=A, 2=B(+AG), 3=C, 4=D, 5=E

    @property
    def HD(self):
        return self.H // self.NH

    @property
    def NG(self):
        return self.W // self.B

    @property
    def T(self):
        return self.S // self.NG

    @property
    def NT(self):
        return self.T // 128

    @property
    def KH(self):
        return self.H // 128

    @property
    def NOS(self):
        return self.H // self.WSP

    @property
    def SCALE(self):
        return 1.0 / float(np.sqrt(self.HD))


def build(cfg: Cfg):
    c = cfg
    assert c.HD == 128 and c.T % 128 == 0 and c.H % c.WSP == 0
    assert c.FF % c.FC == 0 and c.FC % 128 == 0

    nc = bacc.Bacc("TRN2", target_bir_lowering=False, debug=False,
                   num_devices=c.W)
    d = lambda name, shape, dt=F32: nc.dram_tensor(name, shape, dt,
                                                   kind="ExternalInput")
    io = {}
    io["x_in"] = d("x", [c.T, c.H])
    # wq/wk host-relaid as [128, NH*KH*128]: col block h*KH*128 holds the
    # head-h panel in (kk, f) order -> contiguous single-descriptor DMA.
    io["wq_in"] = d("wq", [128, c.NH * c.KH * 128], BF16)
    io["wk_in"] = d("wk", [128, c.NH * c.KH * 128], BF16)
    io["wv_in"] = d("wv", [c.H, c.H], BF16)
    io["wd_in"] = d("wd", [c.H, c.H], BF16)
    io["w1_in"] = d("w1", [c.H, c.FF], BF16)
    io["w2_in"] = d("w2", [c.FF, c.H], BF16)
    io["bq_in"] = d("bq", [1, c.H], BF16)
    io["bk_in"] = d("bk", [1, c.H], BF16)
    io["bd_in"] = d("bd", [1, c.H], BF16)   # holds bv@wd + b_dense
    io["b1_in"] = d("b1", [128, c.FF // 128])
    io["b2_in"] = d("b2", [1, c.H], BF16)
    io["ones_r_in"] = d("ones_r", [1, c.T], BF16)
    io["ones_c_in"] = d("ones_c", [128, 1], BF16)
    io["cos_in"] = d("cosT", [128, c.T])
    io["sins_in"] = d("sinsT", [128, c.T])
    io["mask_in"] = d("maskT", [c.S, c.T], BF16)
    io["out_ext"] = nc.dram_tensor("out", [c.T, c.H], F32, kind="ExternalOutput")
    io["groups"] = [[g * c.NG + r for r in range(c.NG)] for g in range(c.B)]

    with tile.TileContext(nc) as tc:
        _body(nc, tc, c, io)
    nc.compile()
    return nc


def _body(nc, tc, c, io):
    x_in, out_ext = io["x_in"], io["out_ext"]
    NT, KH, NH, T, H = c.NT, c.KH, c.NH, c.T, c.H
    WSP, NOS = c.WSP, c.NOS
    SKT = c.S // 128
    AXX = mybir.AxisListType.X

    # ---------- persistent pools ----------
    const = tc.alloc_tile_pool(name="const", bufs=1)
    ident = const.tile([128, 128], F32, tag="ident", name="ident")
    make_identity(nc, ident[:])
    ident_bf = const.tile([128, 128], BF16, tag="identbf", name="identbf")
    nc.vector.tensor_copy(ident_bf[:], ident[:])
    ones_r = const.tile([1, T], BF16, tag="ones_r", name="ones_r")
    nc.sync.dma_start(ones_r[:], io["ones_r_in"].ap()[:])
    ones_c = const.tile([128, 1], BF16, tag="ones_c", name="ones_c")
    nc.sync.dma_start(ones_c[:], io["ones_c_in"].ap()[:])
    epsap = const.tile([128, 1], F32, tag="epsap", name="epsap")
    nc.gpsimd.memset(epsap[:], c.EPS)
    b1_sb = const.tile([128, c.FF // 128], F32, tag="b1", name="b1")
    nc.sync.dma_start(b1_sb[:], io["b1_in"].ap()[:])
    cos_sb = const.tile([128, T], F32, tag="cos", name="cos")
    nc.sync.dma_start(cos_sb[:], io["cos_in"].ap()[:])
    sins_sb = const.tile([128, T], F32, tag="sins", name="sins")
    nc.sync.dma_start(sins_sb[:], io["sins_in"].ap()[:])

    stat = tc.alloc_tile_pool(name="stat", bufs=2)
    big = tc.alloc_tile_pool(name="big", bufs=1)
    pp = tc.alloc_tile_pool(name="pp", bufs=1, space="PSUM")
    dram = tc.alloc_tile_pool(name="dram", bufs=1, space="DRAM")

    HC = NH // 2        # heads per K AllGather chunk
    ag_space = maybe_share_collective_output_space("AllGather", io["groups"])
    kT_bnc = [dram.tile([HC * 128, T], BF16, tag="kTb", name=f"kTb{ci}",
                        bufs=2) for ci in range(2)]
    kT_allc = [dram.tile([c.NG * HC * 128, T], BF16, tag="kTall",
                         name=f"kTall{ci}", bufs=2, addr_space=ag_space)
               for ci in range(2)]
    v_bnc = [dram.tile([256, H], BF16, tag="vb", name=f"vb{ci}", bufs=2)
             for ci in range(2)]
    v_alls = [dram.tile([c.NG * 256, H], BF16, tag="vall", name=f"vall{ci}",
                        bufs=2, addr_space=ag_space) for ci in range(2)]

    def ln_tile(src, out, scrpool):
        """LN stats + normalize for one N-layout tile [128, H] -> out."""
        s1 = stat.tile([128, 1], F32, tag="s1", name="s1")
        nc.vector.tensor_reduce(s1[:], src[:], axis=AXX, op=ALU.add)
        scr = scrpool.tile([128, H], F32, tag="lnscr", name="lnscr", bufs=2)
        nc.scalar.square(scr[:], src[:])
        s2 = stat.tile([128, 1], F32, tag="s2", name="s2")
        nc.vector.tensor_reduce(s2[:], scr[:], axis=AXX, op=ALU.add)
        m = stat.tile([128, 1], F32, tag="m", name="m")
        nc.scalar.mul(m[:], s1[:], 1.0 / H)
        msq = stat.tile([128, 1], F32, tag="msq", name="msq")
        nc.vector.scalar_tensor_tensor(msq[:], m[:], 1.0, m[:],
                                       op0=ALU.mult, op1=ALU.mult)
        var = stat.tile([128, 1], F32, tag="var", name="var")
        nc.vector.scalar_tensor_tensor(var[:], s2[:], 1.0 / H, msq[:],
                                       op0=ALU.mult, op1=ALU.subtract)
        std = stat.tile([128, 1], F32, tag="std", name="std")
        nc.scalar.activation(std[:], var[:], AF.Sqrt, bias=epsap[:], scale=1.0)
        rstd = stat.tile([128, 1], F32, tag="rstd", name="rstd")
        nc.vector.reciprocal(rstd[:], std[:])
        negmr = stat.tile([128, 1], F32, tag="negmr", name="negmr")
        nc.vector.scalar_tensor_tensor(negmr[:], m[:], -1.0, rstd[:],
                                       op0=ALU.mult, op1=ALU.mult)
        nc.scalar.activation(out[:], src[:], AF.Identity,
                             bias=negmr[:], scale=rstd[:])

    def transpose_tile(srcN, dstT_list, t):
        """[128tok, H] f32 -> cast into KH bf16 T-layout tiles at column t."""
        for kk in range(KH):
            ps = pp.tile([128, 128], F32, tag="ptr", name="ptr", bufs=1)
            nc.tensor.transpose(ps[:], srcN[:, 128 * kk:128 * (kk + 1)], ident[:])
            nc.vector.tensor_copy(dstT_list[kk][:, 128 * t:128 * (t + 1)], ps[:])

    # ---------- Phase A: LN1 + transpose (x streamed) ----------
    pa = tc.alloc_tile_pool(name="pa", bufs=1)
    xT = [big.tile([128, T], BF16, tag="TT", name=f"xT{kk}", bufs=KH)
          for kk in range(KH)]
    for t in range(NT):
        xt = pa.tile([128, H], F32, tag="ax", name=f"x{t}", bufs=3)
        nc.sync.dma_start(xt[:], x_in.ap()[128 * t:128 * (t + 1), :])
        xh = pa.tile([128, H], F32, tag="axh", name=f"xh{t}", bufs=3)
        ln_tile(xt, xh, pa)
        transpose_tile(xh, xT, t)
    pa.release()

    # ---------- Phase B: K (2 chunked AGs) -> V (tiled AGs) -> Q ----------
    pcd = tc.alloc_tile_pool(name="pcd", bufs=1)   # ctx tiles (live into D)
    pb = tc.alloc_tile_pool(name="pb", bufs=1)
    qT = [big.tile([128, T], BF16, tag="qT", name=f"qT{h}", bufs=NH)
          for h in range(NH)]

    def qk_head(h, w_in, b_in, dst):
        """dst: (dram_ap, row0) or sbuf tile"""
        ps = pp.tile([128, T], F32, tag="mm", name="pqk", bufs=2)
        wt = pb.tile([128, KH * 128], BF16, tag="wqk", name="wqk", bufs=6)
        nc.sync.dma_start(
            wt[:], w_in.ap()[:, h * KH * 128:(h + 1) * KH * 128])
        for kk in range(KH):
            nc.tensor.matmul(ps[:], wt[:, 128 * kk:128 * (kk + 1)], xT[kk][:],
                             start=(kk == 0), stop=False)
        bt = pb.tile([1, 128], BF16, tag="bqk", name="bqk", bufs=2)
        nc.sync.dma_start(bt[:], b_in.ap()[:, 128 * h:128 * (h + 1)])
        nc.tensor.matmul(ps[:], bt[:], ones_r[:], start=False, stop=True)
        tmp = pb.tile([128, T], F32, tag="ropetmp", name="ropetmp", bufs=3)
        nc.vector.scalar_tensor_tensor(tmp[0:64, :], ps[64:128, :], 1.0,
                                       sins_sb[0:64, :], op0=ALU.mult,
                                       op1=ALU.mult)
        nc.vector.scalar_tensor_tensor(tmp[64:128, :], ps[0:64, :], 1.0,
                                       sins_sb[64:128, :], op0=ALU.mult,
                                       op1=ALU.mult)
        qc = pb.tile([128, T], F32, tag="ropeqc", name="ropeqc", bufs=3)
        nc.vector.scalar_tensor_tensor(qc[:], ps[:], 1.0, cos_sb[:],
                                       op0=ALU.mult, op1=ALU.mult)
        if isinstance(dst, tuple):
            res = pb.tile([128, T], BF16, tag="qkres", name="qkres", bufs=3)
            nc.vector.scalar_tensor_tensor(res[:], qc[:], 1.0, tmp[:],
                                           op0=ALU.mult, op1=ALU.add)
            d_ap, row0 = dst
            nc.sync.dma_start(d_ap[row0:row0 + 128, :], res[:])
        else:
            nc.vector.scalar_tensor_tensor(dst[:], qc[:], 1.0, tmp[:],
                                           op0=ALU.mult, op1=ALU.add)

    # K first, AllGather per 8-head chunk so the CC stream starts early.
    for ci in range(2):
        for hl in range(HC):
            qk_head(ci * HC + hl, io["wk_in"], io["bk_in"],
                    (kT_bnc[ci], 128 * hl))
        nc.gpsimd.collective_compute(
            "AllGather", ALU.bypass, ins=[kT_bnc[ci].opt()],
            outs=[kT_allc[ci].opt()], replica_groups=io["groups"])

    # V next so its AGs queue right behind K's chunks (2 chunks of 2 tiles).
    ppv = tc.alloc_tile_pool(name="ppv", bufs=1, space="PSUM")
    pv = tc.alloc_tile_pool(name="pv", bufs=1)
    wv_sb = []
    for kk in range(KH):
        wt = pv.tile([128, H], BF16, tag="wvpan", name="wvpan", bufs=KH)
        nc.sync.dma_start(wt[:], io["wv_in"].ap()[128 * kk:128 * (kk + 1), :])
        wv_sb.append(wt)
    for ci in range(2):
        for tl in range(2):
            t = 2 * ci + tl
            pss = [ppv.tile([128, WSP], F32, tag=f"pvac{o}", name=f"pvac{o}",
                            bufs=1) for o in range(NOS)]
            for kk in range(KH):
                lhs = xT[kk][:, 128 * t:128 * (t + 1)]
                for osp in range(NOS):
                    nc.tensor.matmul(pss[osp][:], lhs,
                                     wv_sb[kk][:, WSP * osp:WSP * (osp + 1)],
                                     start=(kk == 0), stop=(kk == KH - 1))
            for osp in range(NOS):
                vs = pb.tile([128, WSP], BF16, tag="vslice", name="vslice",
                             bufs=3)
                nc.vector.tensor_copy(vs[:], pss[osp][:])
                nc.sync.dma_start(
                    v_bnc[ci][128 * tl:128 * (tl + 1),
                              WSP * osp:WSP * (osp + 1)], vs[:])
        nc.gpsimd.collective_compute(
            "AllGather", ALU.bypass, ins=[v_bnc[ci].opt()],
            outs=[v_alls[ci].opt()], replica_groups=io["groups"])
    pv.release()
    ppv.release()

    # ---------- Phase C: attention, k-token-tile-major (AG-arrival order) ---
    pc = tc.alloc_tile_pool(name="pc", bufs=1)
    ppc = tc.alloc_tile_pool(name="ppc", bufs=1, space="PSUM")
    mask_sb = [pc.tile([128, T], BF16, tag="mask", name=f"mask{m}", bufs=SKT)
               for m in range(SKT)]
    for m in range(SKT):
        nc.sync.dma_start(mask_sb[m][:],
                          io["mask_in"].ap()[128 * m:128 * (m + 1), :])
    ctxacc = [pc.tile([128, T], F32, tag="ctxacc", name=f"ca{h}", bufs=NH)
              for h in range(NH)]
    # softmax denominators packed 4-per-tile at 32-aligned partition bases
    sum_tiles = [pc.tile([128, T], BF16, tag="sumacc", name=f"sumacc{i}",
                         bufs=4) for i in range(4)]
    sslice = lambda h: sum_tiles[h // 4][32 * (h % 4):32 * (h % 4) + 1, :]
    ctx = [None] * NH
    for lm in range(NT):
        for h in range(NH):
            if lm == 0:
                # JIT Q: compute head h+1's Q while head h's rope drains,
                # so the score matmuls never wait on the vector engine.
                if h == 0:
                    qk_head(0, io["wq_in"], io["bq_in"], qT[0])
                if h + 1 < NH:
                    qk_head(h + 1, io["wq_in"], io["bq_in"], qT[h + 1])
            ci, hl = divmod(h, HC)
            # K block panel for (lm, h): [128 hd, r-major 4x128 keys].
            kp = pc.tile([128, c.NG * 128], BF16, tag="kpan", name="kpan",
                         bufs=6)
            nc.gpsimd.dma_start(
                kp[:].rearrange("p (r f) -> p r f", f=128),
                kT_allc[ci].rearrange("(r hh p) t -> p hh r t", hh=HC, p=128)
                [:, hl, :, 128 * lm:128 * (lm + 1)])
            vp = pc.tile([128, c.NG * 128], BF16, tag="vpan", name="vpan",
                         bufs=6)
            nc.sync.dma_start(
                vp[:].rearrange("p (r f) -> p r f", f=128),
                v_alls[lm // 2].rearrange("(r tt p) cc -> p tt r cc",
                                          tt=2, p=128)
                [:, lm % 2, :, 128 * h:128 * (h + 1)])
            ems = []
            for r in range(c.NG):
                m = r * NT + lm
                ps_s = ppc.tile([128, T], F32, tag="pscore", name="pscore",
                                bufs=2)
                nc.tensor.matmul(ps_s[:], kp[:, 128 * r:128 * (r + 1)],
                                 qT[h][:], start=True, stop=False)
                # additive log-mask (0 / -1e5) folded into the psum: one
                # I^T @ M matmul — keeps masking off the vector engine.
                nc.tensor.matmul(ps_s[:], ident_bf[:], mask_sb[m][:],
                                 start=False, stop=True)
                e_m = pc.tile([128, T], BF16, tag="eatt", name="eatt", bufs=6)
                nc.scalar.activation(e_m[:], ps_s[:], AF.Exp, bias=0.0,
                                     scale=c.SCALE)
                ems.append(e_m)
            ps_ctx = ppc.tile([128, T], F32, tag="pctx", name="pctx", bufs=2)
            for r in range(c.NG):
                nc.tensor.matmul(ps_ctx[:], vp[:, 128 * r:128 * (r + 1)],
                                 ems[r][:], start=(r == 0),
                                 stop=(r == c.NG - 1))
            ps_sum = ppc.tile([1, T], F32, tag="psml", name="psml", bufs=1)
            for r in range(c.NG):
                nc.tensor.matmul(ps_sum[:], ones_c[:], ems[r][:],
                                 start=(r == 0), stop=(r == c.NG - 1))
            if lm == 0:
                nc.vector.tensor_copy(ctxacc[h][:], ps_ctx[:])
                nc.vector.tensor_copy(sslice(h), ps_sum[:])
            else:
                nc.vector.scalar_tensor_tensor(ctxacc[h][:], ps_ctx[:], 1.0,
                                               ctxacc[h][:], op0=ALU.mult,
                                               op1=ALU.add)
                nc.vector.scalar_tensor_tensor(sslice(h), ps_sum[:],
                                               1.0, sslice(h),
                                               op0=ALU.mult, op1=ALU.add)
            if lm == NT - 1:
                # normalize head h right away — overlaps later heads' scores
                rsum = stat.tile([1, T], F32, tag="rsum", name="rsum")
                nc.vector.reciprocal(rsum[:], sslice(h))
                rrep = stat.tile([128, T], F32, tag="rsumrep", name="rsumrep")
                nc.gpsimd.partition_broadcast(rrep[:], rsum[:])
                cn = pcd.tile([128, T], BF16, tag="ctx", name=f"ctx{h}",
                              bufs=NH)
                nc.vector.scalar_tensor_tensor(cn[:], ctxacc[h][:], 1.0,
                                               rrep[:], op0=ALU.mult,
                                               op1=ALU.mult)
                ctx[h] = cn
    ppc.release()
    pc.release()
    pb.release()

    # ---------- Phase D: dense + residual, LN2, transpose ----------
    pd = tc.alloc_tile_pool(name="pd", bufs=1)
    hid_b = dram.tile([T, H], F32, tag="hidb", name="hidb")
    ppd = tc.alloc_tile_pool(name="ppd", bufs=1, space="PSUM")
    wd_sb = []
    for kk in range(KH):
        wt = pd.tile([128, H], BF16, tag="wdpan", name="wdpan", bufs=KH)
        nc.sync.dma_start(wt[:], io["wd_in"].ap()[128 * kk:128 * (kk + 1), :])
        wd_sb.append(wt)
    bts = []
    for osp in range(NOS):
        bt = pd.tile([1, WSP], BF16, tag="bdsl", name="bdsl", bufs=NOS)
        nc.sync.dma_start(bt[:], io["bd_in"].ap()[:, WSP * osp:WSP * (osp + 1)])
        bts.append(bt)
    for t in range(NT):
        pss = [ppd.tile([128, WSP], F32, tag=f"pdac{o}", name=f"pdac{o}",
                        bufs=1) for o in range(NOS)]
        for kk in range(KH):
            lhs = ctx[kk][:, 128 * t:128 * (t + 1)]
            for osp in range(NOS):
                nc.tensor.matmul(pss[osp][:], lhs,
                                 wd_sb[kk][:, WSP * osp:WSP * (osp + 1)],
                                 start=(kk == 0), stop=False)
        for osp in range(NOS):
            nc.tensor.matmul(pss[osp][:], ones_r[:, 0:128], bts[osp][:],
                             start=False, stop=True)
            xs = pd.tile([128, WSP], F32, tag="xsl", name="xsl", bufs=3)
            nc.sync.dma_start(
                xs[:], x_in.ap()[128 * t:128 * (t + 1),
                                 WSP * osp:WSP * (osp + 1)])
            hs = pd.tile([128, WSP], F32, tag="hsl", name="hsl", bufs=3)
            nc.vector.scalar_tensor_tensor(hs[:], pss[osp][:], 1.0, xs[:],
                                           op0=ALU.mult, op1=ALU.add)
            nc.sync.dma_start(
                hid_b[128 * t:128 * (t + 1), WSP * osp:WSP * (osp + 1)], hs[:])
    ppd.release()
    pd.release()
    pcd.release()

    pdh = tc.alloc_tile_pool(name="pdh", bufs=1)
    hT = [big.tile([128, T], BF16, tag="TT", name=f"hT{kk}", bufs=KH)
          for kk in range(KH)]
    for t in range(NT):
        ht = pdh.tile([128, H], F32, tag="dh", name=f"hid{t}", bufs=3)
        nc.sync.dma_start(ht[:], hid_b[128 * t:128 * (t + 1), :])
        hh = pdh.tile([128, H], F32, tag="dhh", name=f"hh{t}", bufs=3)
        ln_tile(ht, hh, pdh)
        transpose_tile(hh, hT, t)
    pdh.release()

    # ---------- Phase E: fused MLP ----------
    pe = tc.alloc_tile_pool(name="pe", bufs=1)
    ppe2 = tc.alloc_tile_pool(name="ppe2", bufs=1, space="PSUM")
    NFC = c.FF // c.FC
    FCT = c.FC // 128
    out_t = [big.tile([128, H], F32, tag="bigH", name=f"out{t}", bufs=4)
             for t in range(NT)]
    for f in range(NFC):
        gT = []
        for mm in range(FCT):
            fglob = f * FCT + mm
            w1t = pe.tile([128, KH * 128], BF16, tag="w1pan", name="w1pan",
                          bufs=4)
            nc.sync.dma_start(
                w1t[:].rearrange("p (kk f) -> p kk f", f=128),
                io["w1_in"].ap()[:, 128 * fglob:128 * (fglob + 1)]
                .rearrange("(kk p) f -> p kk f", p=128))
            ps = pp.tile([128, T], F32, tag="mm", name="pm1", bufs=2)
            for kk in range(KH):
                nc.tensor.matmul(ps[:], w1t[:, 128 * kk:128 * (kk + 1)],
                                 hT[kk][:], start=(kk == 0),
                                 stop=(kk == KH - 1))
            g = pe.tile([128, T], BF16, tag="gT", name="gT", bufs=FCT + 4)
            if c.sim_gelu:
                a = pe.tile([128, T], F32, tag="ga", name="ga", bufs=2)
                nc.scalar.activation(a[:], ps[:], AF.Identity,
                                     bias=b1_sb[:, fglob:fglob + 1], scale=1.0)
                sg = pe.tile([128, T], F32, tag="gsg", name="gsg", bufs=2)
                nc.scalar.activation(sg[:], a[:], AF.Sigmoid, bias=0.0,
                                     scale=1.702)
                nc.vector.scalar_tensor_tensor(g[:], a[:], 1.0, sg[:],
                                               op0=ALU.mult, op1=ALU.mult)
            else:
                nc.scalar.activation(g[:], ps[:], AF.Gelu,
                                     bias=b1_sb[:, fglob:fglob + 1], scale=1.0)
            gT.append(g)
        w2_sb = []
        for kf in range(FCT):
            wt = pe.tile([128, H], BF16, tag="w2pan", name="w2pan",
                         bufs=FCT + 4)
            nc.sync.dma_start(
                wt[:], io["w2_in"].ap()[128 * (f * FCT + kf):
                                        128 * (f * FCT + kf + 1), :])
            w2_sb.append(wt)
        if f == 0:
            b2s = []
            for osp in range(NOS):
                bt = pe.tile([1, WSP], BF16, tag="b2sl", name="b2sl", bufs=NOS)
                nc.sync.dma_start(
                    bt[:], io["b2_in"].ap()[:, WSP * osp:WSP * (osp + 1)])
                b2s.append(bt)
        for t in range(NT):
            pss = [ppe2.tile([128, WSP], F32, tag=f"pmac{o}", name=f"pmac{o}",
                             bufs=1) for o in range(NOS)]
            for kf in range(FCT):
                lhs = gT[kf][:, 128 * t:128 * (t + 1)]
                for osp in range(NOS):
                    nc.tensor.matmul(pss[osp][:], lhs,
                                     w2_sb[kf][:, WSP * osp:WSP * (osp + 1)],
                                     start=(kf == 0),
                                     stop=(kf == FCT - 1 and f != 0))
            for osp in range(NOS):
                osl = out_t[t][:, WSP * osp:WSP * (osp + 1)]
                if f == 0:
                    nc.tensor.matmul(pss[osp][:], ones_r[:, 0:128], b2s[osp][:],
                                     start=False, stop=True)
                    hsl = pe.tile([128, WSP], F32, tag="hres", name="hres",
                                  bufs=3)
                    nc.sync.dma_start(
                        hsl[:], hid_b[128 * t:128 * (t + 1),
                                      WSP * osp:WSP * (osp + 1)])
                    nc.vector.scalar_tensor_tensor(osl, pss[osp][:], 1.0,
                                                   hsl[:], op0=ALU.mult,
                                                   op1=ALU.add)
                else:
                    nc.vector.scalar_tensor_tensor(osl, pss[osp][:], 1.0, osl,
                                                   op0=ALU.mult, op1=ALU.add)
    ppe2.release()
    pe.release()

    # ---------- Phase F: output ----------
    for t in range(NT):
        nc.sync.dma_start(out_ext.ap()[128 * t:128 * (t + 1), :], out_t[t][:])

    for p in (pp, dram, big, stat, const):
        p.release()


# ---------------- host side ----------------

def prepare_in_maps(c: Cfg, inputs):
    f32 = np.float32
    hs = np.asarray(inputs["hidden_states"], f32)
    ln1_g = np.asarray(inputs["ln1_g"], f32)
    ln1_b = np.asarray(inputs["ln1_b"], f32)
    w_qkv = np.asarray(inputs["w_qkv"], f32)
    b_qkv = np.asarray(inputs["b_qkv"], f32)
    w_dense = np.asarray(inputs["w_dense"], f32)
    b_dense = np.asarray(inputs["b_dense"], f32)
    ln2_g = np.asarray(inputs["ln2_g"], f32)
    ln2_b = np.asarray(inputs["ln2_b"], f32)
    w1 = np.asarray(inputs["w1"], f32)
    b1 = np.asarray(inputs["b1"], f32)
    w2 = np.asarray(inputs["w2"], f32)
    b2 = np.asarray(inputs["b2"], f32)

    H, NH, HD, FF = c.H, c.NH, c.HD, c.FF
    cols = np.concatenate([np.arange(h * 3 * HD, h * 3 * HD + HD)
                           for h in range(NH)])
    wg = ln1_g[:, None] * w_qkv
    wq_f, wk_f, wv_f = wg[:, cols], wg[:, cols + HD], wg[:, cols + 2 * HD]
    bfull = ln1_b @ w_qkv + b_qkv
    bq_f, bk_f, bv_f = bfull[cols], bfull[cols + HD], bfull[cols + 2 * HD]
    bd_f = bv_f @ w_dense + b_dense          # v-bias folded through attention
    w1_f = ln2_g[:, None] * w1
    b1_f = ln2_b @ w1 + b1

    inv = 1.0 / (10000.0 ** (np.arange(0, HD, 2, dtype=f32) / HD))
    pos = np.arange(c.S, dtype=f32)
    frq = np.einsum('i,j->ij', pos, inv)
    emb = np.concatenate([frq, frq], axis=-1)
    cos_full = np.cos(emb).T.astype(f32)
    sin_full = np.sin(emb).T.astype(f32)
    sins_full = sin_full.copy()
    sins_full[:HD // 2] *= -1.0

    bf = lambda a: np.ascontiguousarray(a.astype(BF))
    KH = H // 128
    # [H, H] head-major -> [128, NH*KH*128]: col block h*KH*128+(kk*128+f)
    # = w[kk*128+p, h*128+f] (contiguous per-head panel for one-descriptor DMA)
    relay = lambda w: (w.reshape(KH, 128, NH, 128).transpose(1, 2, 0, 3)
                       .reshape(128, NH * KH * 128))
    wqT, wkT = relay(wq_f), relay(wk_f)
    in_maps = []
    for i in range(c.W):
        b, g = i // c.NG, i % c.NG
        t0 = g * c.T
        qpos = np.arange(t0, t0 + c.T)
        kpos = np.arange(c.S)
        mask = np.where(kpos[:, None] <= qpos[None, :],
                        np.float32(0.0), np.float32(-1e5)).astype(BF)
        in_maps.append({
            "x": np.ascontiguousarray(hs[b, t0:t0 + c.T, :]),
            "wq": bf(wqT), "wk": bf(wkT), "wv": bf(wv_f),
            "wd": bf(w_dense), "w1": bf(w1_f), "w2": bf(w2),
            "bq": bf(bq_f.reshape(1, H)), "bk": bf(bk_f.reshape(1, H)),
            "bd": bf(bd_f.reshape(1, H)),
            "b1": np.ascontiguousarray(b1_f.reshape(FF // 128, 128).T),
            "b2": bf(b2.reshape(1, H)),
            "ones_r": np.ones((1, c.T), BF),
            "ones_c": np.ones((128, 1), BF),
            "cosT": np.ascontiguousarray(cos_full[:, t0:t0 + c.T]),
            "sinsT": np.ascontiguousarray(sins_full[:, t0:t0 + c.T]),
            "maskT": np.ascontiguousarray(mask),
        })
    return in_maps


def assemble_output(c: Cfg, results):
    out = np.empty((c.B, c.S, c.H), np.float32)
    for i in range(c.W):
        b, g = i // c.NG, i % c.NG
        out[b, g * c.T:(g + 1) * c.T, :] = results[i]["out"]
    return out


def run(nc, c: Cfg, inputs, trace=False, **kw):
    in_maps = prepare_in_maps(c, inputs)
    last = None
    for attempt in range(3):
        try:
            res = bass_utils.run_bass_kernel_spmd(
                nc, in_maps, core_ids=list(range(c.W)), trace=trace, **kw)
            return assemble_output(c, res.results), res
        except Exception as e:
            last = e
            print(f"run attempt {attempt} failed: {type(e).__name__}: {e}",
                  file=sys.stderr)
    raise last


# ======================= harness entry point =======================

_CACHE = {}


def kernel(**inputs):
    """Full-input entry: shard, compile (cached), run on 8 cores, gather."""
    c = Cfg()
    if "nc" not in _CACHE:
        _CACHE["nc"] = build(c)
    out, _ = run(_CACHE["nc"], c, inputs, trace=False)
    return out



# revision 29
# speedup vs baseline: 1.0610x; 1.0448x over previous
"""Fused GPT transformer layer on 8 trn2 cores — token-parallel + KV AllGather.

Sharding: core i owns 512 contiguous tokens (cores 0-3 batch 0, 4-7 batch 1).
Per core: LN1 -> QKV (+RoPE) local; AllGather K^T,V within 4-core group;
masked full-key attention (softmax without max-subtraction — safe since
scores ~ N(0,1) for LN'd inputs); dense+residual, LN2, fused chunked MLP all
local. Host gathers per-core outputs.

v3: all matmul operands bf16 (FWL weight loads, half DMA/SBUF), fp32 PSUM
accumulation and fp32 LN/residual/softmax-normalization. Order K -> AG(K) ->
Q -> V -> AG(V) -> attention so local compute hides both collectives.

Layouts:  "N" = [token-partition, feature-free]; "T" = [feature-part, token-free].
"""
import sys
if '/opt/trn_rl_repo' not in sys.path:
    sys.path.insert(0, '/opt/trn_rl_repo')

from dataclasses import dataclass

import numpy as np
import ml_dtypes

import concourse.bass as bass
import concourse.bacc as bacc
import concourse.tile as tile
import concourse.mybir as mybir
from concourse import bass_utils
from concourse.masks import make_identity
from concourse.replica_groups import maybe_share_collective_output_space

F32 = mybir.dt.float32
BF16 = mybir.dt.bfloat16
U8 = mybir.dt.uint8
AF = mybir.ActivationFunctionType
ALU = mybir.AluOpType
BF = ml_dtypes.bfloat16


@dataclass
class Cfg:
    B: int = 2
    S: int = 2048
    H: int = 2048
    NH: int = 16
    FF: int = 8192
    W: int = 8           # total cores
    FC: int = 1024       # FF chunk for fused MLP
    WSP: int = 512       # weight panel span (moving free dim for N-layout mms)
    EPS: float = 1e-5
    sim_gelu: bool = False
    phase_limit: int = 99   # 1=A, 2=B(+AG), 3=C, 4=D, 5=E

    @property
    def HD(self):
        return self.H // self.NH

    @property
    def NG(self):
        return self.W // self.B

    @property
    def T(self):
        return self.S // self.NG

    @property
    def NT(self):
        return self.T // 128

    @property
    def KH(self):
        return self.H // 128

    @property
    def NOS(self):
        return self.H // self.WSP

    @property
    def SCALE(self):
        return 1.0 / float(np.sqrt(self.HD))


def build(cfg: Cfg):
    c = cfg
    assert c.HD == 128 and c.T % 128 == 0 and c.H % c.WSP == 0
    assert c.FF % c.FC == 0 and c.FC % 128 == 0

    nc = bacc.Bacc("TRN2", target_bir_lowering=False, debug=False,
                   num_devices=c.W)
    d = lambda name, shape, dt=F32: nc.dram_tensor(name, shape, dt,
                                                   kind="ExternalInput")
    io = {}
    io["x_in"] = d("x", [c.T, c.H])
    # wq/wk host-relaid as [128, NH*KH*128]: col block h*KH*128 holds the
    # head-h panel in (kk, f) order -> contiguous single-descriptor DMA.
    io["wq_in"] = d("wq", [128, c.NH * c.KH * 128], BF16)
    io["wk_in"] = d("wk", [128, c.NH * c.KH * 128], BF16)
    io["wv_in"] = d("wv", [c.H, c.H], BF16)
    io["wd_in"] = d("wd", [c.H, c.H], BF16)
    io["w1_in"] = d("w1", [c.H, c.FF], BF16)
    io["w2_in"] = d("w2", [c.FF, c.H], BF16)
    io["bq_in"] = d("bq", [1, c.H], BF16)
    io["bk_in"] = d("bk", [1, c.H], BF16)
    io["bd_in"] = d("bd", [1, c.H], BF16)   # holds bv@wd + b_dense
    io["b1_in"] = d("b1", [128, c.FF // 128])
    io["b2_in"] = d("b2", [1, c.H], BF16)
    io["ones_r_in"] = d("ones_r", [1, c.T], BF16)
    io["ones_c_in"] = d("ones_c", [128, 1], BF16)
    io["cos_in"] = d("cosT", [128, c.T])
    io["sins_in"] = d("sinsT", [128, c.T])
    io["mask_in"] = d("maskT", [c.S, c.T], BF16)
    io["out_ext"] = nc.dram_tensor("out", [c.T, c.H], F32, kind="ExternalOutput")
    io["groups"] = [[g * c.NG + r for r in range(c.NG)] for g in range(c.B)]

    with tile.TileContext(nc) as tc:
        _body(nc, tc, c, io)
    nc.compile()
    return nc


def _body(nc, tc, c, io):
    x_in, out_ext = io["x_in"], io["out_ext"]
    NT, KH, NH, T, H = c.NT, c.KH, c.NH, c.T, c.H
    WSP, NOS = c.WSP, c.NOS
    SKT = c.S // 128
    AXX = mybir.AxisListType.X

    # ---------- persistent pools ----------
    const = tc.alloc_tile_pool(name="const", bufs=1)
    ident = const.tile([128, 128], F32, tag="ident", name="ident")
    make_identity(nc, ident[:])
    ident_bf = const.tile([128, 128], BF16, tag="identbf", name="identbf")
    nc.vector.tensor_copy(ident_bf[:], ident[:])
    ones_r = const.tile([1, T], BF16, tag="ones_r", name="ones_r")
    nc.sync.dma_start(ones_r[:], io["ones_r_in"].ap()[:])
    ones_c = const.tile([128, 1], BF16, tag="ones_c", name="ones_c")
    nc.sync.dma_start(ones_c[:], io["ones_c_in"].ap()[:])
    epsap = const.tile([128, 1], F32, tag="epsap", name="epsap")
    nc.gpsimd.memset(epsap[:], c.EPS)
    b1_sb = const.tile([128, c.FF // 128], F32, tag="b1", name="b1")
    nc.sync.dma_start(b1_sb[:], io["b1_in"].ap()[:])
    cos_sb = const.tile([128, T], F32, tag="cos", name="cos")
    nc.sync.dma_start(cos_sb[:], io["cos_in"].ap()[:])
    sins_sb = const.tile([128, T], F32, tag="sins", name="sins")
    nc.sync.dma_start(sins_sb[:], io["sins_in"].ap()[:])

    stat = tc.alloc_tile_pool(name="stat", bufs=2)
    big = tc.alloc_tile_pool(name="big", bufs=1)
    pp = tc.alloc_tile_pool(name="pp", bufs=1, space="PSUM")
    dram = tc.alloc_tile_pool(name="dram", bufs=1, space="DRAM")

    HC = NH // 2        # heads per K AllGather chunk
    ag_space = maybe_share_collective_output_space("AllGather", io["groups"])
    kT_bnc = [dram.tile([HC * 128, T], BF16, tag="kTb", name=f"kTb{ci}",
                        bufs=2) for ci in range(2)]
    kT_allc = [dram.tile([c.NG * HC * 128, T], BF16, tag="kTall",
                         name=f"kTall{ci}", bufs=2, addr_space=ag_space)
               for ci in range(2)]
    v_bnc = [dram.tile([256, H], BF16, tag="vb", name=f"vb{ci}", bufs=2)
             for ci in range(2)]
    v_alls = [dram.tile([c.NG * 256, H], BF16, tag="vall", name=f"vall{ci}",
                        bufs=2, addr_space=ag_space) for ci in range(2)]

    def ln_tile(src, out, scrpool):
        """LN stats + normalize for one N-layout tile [128, H] -> out."""
        s1 = stat.tile([128, 1], F32, tag="s1", name="s1")
        nc.vector.tensor_reduce(s1[:], src[:], axis=AXX, op=ALU.add)
        scr = scrpool.tile([128, H], F32, tag="lnscr", name="lnscr", bufs=2)
        nc.scalar.square(scr[:], src[:])
        s2 = stat.tile([128, 1], F32, tag="s2", name="s2")
        nc.vector.tensor_reduce(s2[:], scr[:], axis=AXX, op=ALU.add)
        m = stat.tile([128, 1], F32, tag="m", name="m")
        nc.scalar.mul(m[:], s1[:], 1.0 / H)
        msq = stat.tile([128, 1], F32, tag="msq", name="msq")
        nc.vector.scalar_tensor_tensor(msq[:], m[:], 1.0, m[:],
                                       op0=ALU.mult, op1=ALU.mult)
        var = stat.tile([128, 1], F32, tag="var", name="var")
        nc.vector.scalar_tensor_tensor(var[:], s2[:], 1.0 / H, msq[:],
                                       op0=ALU.mult, op1=ALU.subtract)
        std = stat.tile([128, 1], F32, tag="std", name="std")
        nc.scalar.activation(std[:], var[:], AF.Sqrt, bias=epsap[:], scale=1.0)
        rstd = stat.tile([128, 1], F32, tag="rstd", name="rstd")
        nc.vector.reciprocal(rstd[:], std[:])
        negmr = stat.tile([128, 1], F32, tag="negmr", name="negmr")
        nc.vector.scalar_tensor_tensor(negmr[:], m[:], -1.0, rstd[:],
                                       op0=ALU.mult, op1=ALU.mult)
        nc.scalar.activation(out[:], src[:], AF.Identity,
                             bias=negmr[:], scale=rstd[:])

    def transpose_tile(srcN, dstT_list, t):
        """[128tok, H] f32 -> cast into KH bf16 T-layout tiles at column t."""
        for kk in range(KH):
            ps = pp.tile([128, 128], F32, tag="ptr", name="ptr", bufs=1)
            nc.tensor.transpose(ps[:], srcN[:, 128 * kk:128 * (kk + 1)], ident[:])
            nc.vector.tensor_copy(dstT_list[kk][:, 128 * t:128 * (t + 1)], ps[:])

    # ---------- Phase A: LN1 + transpose (x streamed) ----------
    pa = tc.alloc_tile_pool(name="pa", bufs=1)
    xT = [big.tile([128, T], BF16, tag="TT", name=f"xT{kk}", bufs=KH)
          for kk in range(KH)]
    for t in range(NT):
        xt = pa.tile([128, H], F32, tag="ax", name=f"x{t}", bufs=3)
        nc.sync.dma_start(xt[:], x_in.ap()[128 * t:128 * (t + 1), :])
        xh = pa.tile([128, H], F32, tag="axh", name=f"xh{t}", bufs=3)
        ln_tile(xt, xh, pa)
        transpose_tile(xh, xT, t)
    pa.release()

    # ---------- Phase B: K (2 chunked AGs) -> V (tiled AGs) -> Q ----------
    pcd = tc.alloc_tile_pool(name="pcd", bufs=1)   # ctx tiles (live into D)
    pb = tc.alloc_tile_pool(name="pb", bufs=1)
    qT = [big.tile([128, T], BF16, tag="qT", name=f"qT{h}", bufs=NH)
          for h in range(NH)]

    def qk_head(h, w_in, b_in, dst):
        """dst: (dram_ap, row0) or sbuf tile"""
        ps = pp.tile([128, T], F32, tag="mm", name="pqk", bufs=2)
        wt = pb.tile([128, KH * 128], BF16, tag="wqk", name="wqk", bufs=4)
        nc.sync.dma_start(
            wt[:], w_in.ap()[:, h * KH * 128:(h + 1) * KH * 128])
        for kk in range(KH):
            nc.tensor.matmul(ps[:], wt[:, 128 * kk:128 * (kk + 1)], xT[kk][:],
                             start=(kk == 0), stop=False)
        bt = pb.tile([1, 128], BF16, tag="bqk", name="bqk", bufs=2)
        nc.sync.dma_start(bt[:], b_in.ap()[:, 128 * h:128 * (h + 1)])
        nc.tensor.matmul(ps[:], bt[:], ones_r[:], start=False, stop=True)
        tmp = pb.tile([128, T], F32, tag="ropetmp", name="ropetmp", bufs=3)
        nc.vector.scalar_tensor_tensor(tmp[0:64, :], ps[64:128, :], 1.0,
                                       sins_sb[0:64, :], op0=ALU.mult,
                                       op1=ALU.mult)
        nc.vector.scalar_tensor_tensor(tmp[64:128, :], ps[0:64, :], 1.0,
                                       sins_sb[64:128, :], op0=ALU.mult,
                                       op1=ALU.mult)
        qc = pb.tile([128, T], F32, tag="ropeqc", name="ropeqc", bufs=3)
        nc.vector.scalar_tensor_tensor(qc[:], ps[:], 1.0, cos_sb[:],
                                       op0=ALU.mult, op1=ALU.mult)
        if isinstance(dst, tuple):
            res = pb.tile([128, T], BF16, tag="qkres", name="qkres", bufs=3)
            nc.vector.scalar_tensor_tensor(res[:], qc[:], 1.0, tmp[:],
                                           op0=ALU.mult, op1=ALU.add)
            d_ap, row0 = dst
            nc.sync.dma_start(d_ap[row0:row0 + 128, :], res[:])
        else:
            nc.vector.scalar_tensor_tensor(dst[:], qc[:], 1.0, tmp[:],
                                           op0=ALU.mult, op1=ALU.add)

    # K first, AllGather per 8-head chunk so the CC stream starts early.
    for ci in range(2):
        for hl in range(HC):
            qk_head(ci * HC + hl, io["wk_in"], io["bk_in"],
                    (kT_bnc[ci], 128 * hl))
        nc.gpsimd.collective_compute(
            "AllGather", ALU.bypass, ins=[kT_bnc[ci].opt()],
            outs=[kT_allc[ci].opt()], replica_groups=io["groups"])

    # V next so its AGs queue right behind K's chunks (2 chunks of 2 tiles).
    ppv = tc.alloc_tile_pool(name="ppv", bufs=1, space="PSUM")
    pv = tc.alloc_tile_pool(name="pv", bufs=1)
    wv_sb = []
    for kk in range(KH):
        wt = pv.tile([128, H], BF16, tag="wvpan", name="wvpan", bufs=KH)
        nc.sync.dma_start(wt[:], io["wv_in"].ap()[128 * kk:128 * (kk + 1), :])
        wv_sb.append(wt)
    for ci in range(2):
        for tl in range(2):
            t = 2 * ci + tl
            pss = [ppv.tile([128, WSP], F32, tag=f"pvac{o}", name=f"pvac{o}",
                            bufs=1) for o in range(NOS)]
            for kk in range(KH):
                lhs = xT[kk][:, 128 * t:128 * (t + 1)]
                for osp in range(NOS):
                    nc.tensor.matmul(pss[osp][:], lhs,
                                     wv_sb[kk][:, WSP * osp:WSP * (osp + 1)],
                                     start=(kk == 0), stop=(kk == KH - 1))
            for osp in range(NOS):
                vs = pb.tile([128, WSP], BF16, tag="vslice", name="vslice",
                             bufs=3)
                nc.vector.tensor_copy(vs[:], pss[osp][:])
                nc.sync.dma_start(
                    v_bnc[ci][128 * tl:128 * (tl + 1),
                              WSP * osp:WSP * (osp + 1)], vs[:])
        nc.gpsimd.collective_compute(
            "AllGather", ALU.bypass, ins=[v_bnc[ci].opt()],
            outs=[v_alls[ci].opt()], replica_groups=io["groups"])
    pv.release()
    ppv.release()

    # ---------- Phase C: attention, per-head with JIT-Q + deferred ctx ----
    pc = tc.alloc_tile_pool(name="pc", bufs=1)
    ppc = tc.alloc_tile_pool(name="ppc", bufs=1, space="PSUM")
    mask_sb = [pc.tile([128, T], BF16, tag="mask", name=f"mask{m}", bufs=SKT)
               for m in range(SKT)]
    for m in range(SKT):
        nc.sync.dma_start(mask_sb[m][:],
                          io["mask_in"].ap()[128 * m:128 * (m + 1), :])
    SKEW = 1

    def scores_head(h):
        ci, hl = divmod(h, HC)
        kpan = pc.tile([128, c.NG * T], BF16, tag="kpan", name="kpan", bufs=2)
        for r in range(c.NG):
            nc.gpsimd.dma_start(
                kpan[:, r * T:(r + 1) * T],
                kT_allc[ci][r * HC * 128 + 128 * hl:
                            r * HC * 128 + 128 * (hl + 1), :])
        ems = []
        for m in range(SKT):
            ps_s = ppc.tile([128, T], F32, tag="pscore", name="pscore", bufs=2)
            nc.tensor.matmul(ps_s[:], kpan[:, 128 * m:128 * (m + 1)],
                             qT[h][:], start=True, stop=True)
            e_m = pc.tile([128, T], BF16, tag="eatt", name="eatt", bufs=4)
            nc.scalar.activation(e_m[:], ps_s[:], AF.Exp, bias=0.0,
                                 scale=c.SCALE)
            em2 = pc.tile([128, T], BF16, tag="eatt2", name="eatt2",
                          bufs=(SKEW + 1) * SKT + 2)
            nc.vector.scalar_tensor_tensor(em2[:], e_m[:], 1.0, mask_sb[m][:],
                                           op0=ALU.mult, op1=ALU.mult)
            ems.append(em2)
        return ems

    def ctx_head(h, ems):
        vpan = [pc.tile([128, c.NG * 128], BF16, tag="vpan", name="vpan",
                        bufs=2 * NT) for _ in range(NT)]
        for lm in range(NT):
            nc.sync.dma_start(
                vpan[lm][:].rearrange("p (r f) -> p r f", f=128),
                v_alls[lm // 2].rearrange("(r tt p) cc -> p tt r cc",
                                          tt=2, p=128)
                [:, lm % 2, :, 128 * h:128 * (h + 1)])
        ps_ctx = ppc.tile([128, T], F32, tag="pctx", name="pctx", bufs=2)
        nmm = 0
        for lm in range(NT):          # chunk-major: V chunk 1 needed last
            for r in range(c.NG):
                nc.tensor.matmul(ps_ctx[:], vpan[lm][:, 128 * r:128 * (r + 1)],
                                 ems[r * NT + lm][:], start=(nmm == 0),
                                 stop=(nmm == SKT - 1))
                nmm += 1
        ps_sum = ppc.tile([1, T], F32, tag="psml", name="psml", bufs=1)
        for m in range(SKT):
            nc.tensor.matmul(ps_sum[:], ones_c[:], ems[m][:],
                             start=(m == 0), stop=(m == SKT - 1))
        rsum = stat.tile([1, T], F32, tag="rsum", name="rsum")
        nc.vector.reciprocal(rsum[:], ps_sum[:])
        rrep = stat.tile([128, T], F32, tag="rsumrep", name="rsumrep")
        nc.gpsimd.partition_broadcast(rrep[:], rsum[:])
        cn = pcd.tile([128, T], BF16, tag="ctx", name=f"ctx{h}", bufs=NH)
        nc.vector.scalar_tensor_tensor(cn[:], ps_ctx[:], 1.0, rrep[:],
                                       op0=ALU.mult, op1=ALU.mult)
        return cn

    ctx = [None] * NH
    pend = []
    qk_head(0, io["wq_in"], io["bq_in"], qT[0])
    for h in range(NH):
        if h + 1 < NH:
            qk_head(h + 1, io["wq_in"], io["bq_in"], qT[h + 1])
        pend.append((h, scores_head(h)))
        if len(pend) > SKEW:
            hh, ems = pend.pop(0)
            ctx[hh] = ctx_head(hh, ems)
    while pend:
        hh, ems = pend.pop(0)
        ctx[hh] = ctx_head(hh, ems)
    ppc.release()
    pc.release()
    pb.release()

    # ---------- Phase D: dense + residual, LN2, transpose ----------
    pd = tc.alloc_tile_pool(name="pd", bufs=1)
    hid_b = dram.tile([T, H], F32, tag="hidb", name="hidb")
    ppd = tc.alloc_tile_pool(name="ppd", bufs=1, space="PSUM")
    wd_sb = []
    for kk in range(KH):
        wt = pd.tile([128, H], BF16, tag="wdpan", name="wdpan", bufs=KH)
        nc.sync.dma_start(wt[:], io["wd_in"].ap()[128 * kk:128 * (kk + 1), :])
        wd_sb.append(wt)
    bts = []
    for osp in range(NOS):
        bt = pd.tile([1, WSP], BF16, tag="bdsl", name="bdsl", bufs=NOS)
        nc.sync.dma_start(bt[:], io["bd_in"].ap()[:, WSP * osp:WSP * (osp + 1)])
        bts.append(bt)
    for t in range(NT):
        pss = [ppd.tile([128, WSP], F32, tag=f"pdac{o}", name=f"pdac{o}",
                        bufs=1) for o in range(NOS)]
        for kk in range(KH):
            lhs = ctx[kk][:, 128 * t:128 * (t + 1)]
            for osp in range(NOS):
                nc.tensor.matmul(pss[osp][:], lhs,
                                 wd_sb[kk][:, WSP * osp:WSP * (osp + 1)],
                                 start=(kk == 0), stop=False)
        for osp in range(NOS):
            nc.tensor.matmul(pss[osp][:], ones_r[:, 0:128], bts[osp][:],
                             start=False, stop=True)
            xs = pd.tile([128, WSP], F32, tag="xsl", name="xsl", bufs=3)
            nc.sync.dma_start(
                xs[:], x_in.ap()[128 * t:128 * (t + 1),
                                 WSP * osp:WSP * (osp + 1)])
            hs = pd.tile([128, WSP], F32, tag="hsl", name="hsl", bufs=3)
            nc.vector.scalar_tensor_tensor(hs[:], pss[osp][:], 1.0, xs[:],
                                           op0=ALU.mult, op1=ALU.add)
            nc.sync.dma_start(
                hid_b[128 * t:128 * (t + 1), WSP * osp:WSP * (osp + 1)], hs[:])
    ppd.release()
    pd.release()
    pcd.release()

    pdh = tc.alloc_tile_pool(name="pdh", bufs=1)
    hT = [big.tile([128, T], BF16, tag="TT", name=f"hT{kk}", bufs=KH)
          for kk in range(KH)]
    for t in range(NT):
        ht = pdh.tile([128, H], F32, tag="dh", name=f"hid{t}", bufs=3)
        nc.sync.dma_start(ht[:], hid_b[128 * t:128 * (t + 1), :])
        hh = pdh.tile([128, H], F32, tag="dhh", name=f"hh{t}", bufs=3)
        ln_tile(ht, hh, pdh)
        transpose_tile(hh, hT, t)
    pdh.release()

    # ---------- Phase E: fused MLP ----------
    pe = tc.alloc_tile_pool(name="pe", bufs=1)
    ppe2 = tc.alloc_tile_pool(name="ppe2", bufs=1, space="PSUM")
    NFC = c.FF // c.FC
    FCT = c.FC // 128
    out_t = [big.tile([128, H], F32, tag="bigH", name=f"out{t}", bufs=4)
             for t in range(NT)]
    for f in range(NFC):
        gT = []
        for mm in range(FCT):
            fglob = f * FCT + mm
            w1t = pe.tile([128, KH * 128], BF16, tag="w1pan", name="w1pan",
                          bufs=4)
            nc.sync.dma_start(
                w1t[:].rearrange("p (kk f) -> p kk f", f=128),
                io["w1_in"].ap()[:, 128 * fglob:128 * (fglob + 1)]
                .rearrange("(kk p) f -> p kk f", p=128))
            ps = pp.tile([128, T], F32, tag="mm", name="pm1", bufs=2)
            for kk in range(KH):
                nc.tensor.matmul(ps[:], w1t[:, 128 * kk:128 * (kk + 1)],
                                 hT[kk][:], start=(kk == 0),
                                 stop=(kk == KH - 1))
            g = pe.tile([128, T], BF16, tag="gT", name="gT", bufs=FCT + 4)
            if c.sim_gelu:
                a = pe.tile([128, T], F32, tag="ga", name="ga", bufs=2)
                nc.scalar.activation(a[:], ps[:], AF.Identity,
                                     bias=b1_sb[:, fglob:fglob + 1], scale=1.0)
                sg = pe.tile([128, T], F32, tag="gsg", name="gsg", bufs=2)
                nc.scalar.activation(sg[:], a[:], AF.Sigmoid, bias=0.0,
                                     scale=1.702)
                nc.vector.scalar_tensor_tensor(g[:], a[:], 1.0, sg[:],
                                               op0=ALU.mult, op1=ALU.mult)
            else:
                nc.scalar.activation(g[:], ps[:], AF.Gelu,
                                     bias=b1_sb[:, fglob:fglob + 1], scale=1.0)
            gT.append(g)
        w2_sb = []
        for kf in range(FCT):
            wt = pe.tile([128, H], BF16, tag="w2pan", name="w2pan",
                         bufs=FCT + 4)
            nc.sync.dma_start(
                wt[:], io["w2_in"].ap()[128 * (f * FCT + kf):
                                        128 * (f * FCT + kf + 1), :])
            w2_sb.append(wt)
        if f == 0:
            b2s = []
            for osp in range(NOS):
                bt = pe.tile([1, WSP], BF16, tag="b2sl", name="b2sl", bufs=NOS)
                nc.sync.dma_start(
                    bt[:], io["b2_in"].ap()[:, WSP * osp:WSP * (osp + 1)])
                b2s.append(bt)
        for t in range(NT):
            pss = [ppe2.tile([128, WSP], F32, tag=f"pmac{o}", name=f"pmac{o}",
                             bufs=1) for o in range(NOS)]
            for kf in range(FCT):
                lhs = gT[kf][:, 128 * t:128 * (t + 1)]
                for osp in range(NOS):
                    nc.tensor.matmul(pss[osp][:], lhs,
                                     w2_sb[kf][:, WSP * osp:WSP * (osp + 1)],
                                     start=(kf == 0),
                                     stop=(kf == FCT - 1 and f != 0))
            for osp in range(NOS):
                osl = out_t[t][:, WSP * osp:WSP * (osp + 1)]
                if f == 0:
                    nc.tensor.matmul(pss[osp][:], ones_r[:, 0:128], b2s[osp][:],
                                     start=False, stop=True)
                    hsl = pe.tile([128, WSP], F32, tag="hres", name="hres",
                                  bufs=3)
                    nc.sync.dma_start(
                        hsl[:], hid_b[128 * t:128 * (t + 1),
                                      WSP * osp:WSP * (osp + 1)])
                    nc.vector.scalar_tensor_tensor(osl, pss[osp][:], 1.0,
                                                   hsl[:], op0=ALU.mult,
                                                   op1=ALU.add)
                else:
                    nc.vector.scalar_tensor_tensor(osl, pss[osp][:], 1.0, osl,
                                                   op0=ALU.mult, op1=ALU.add)
    ppe2.release()
    pe.release()

    # ---------- Phase F: output ----------
    for t in range(NT):
        nc.sync.dma_start(out_ext.ap()[128 * t:128 * (t + 1), :], out_t[t][:])

    for p in (pp, dram, big, stat, const):
        p.release()


# ---------------- host side ----------------

def prepare_in_maps(c: Cfg, inputs):
    f32 = np.float32
    hs = np.asarray(inputs["hidden_states"], f32)
    ln1_g = np.asarray(inputs["ln1_g"], f32)
    ln1_b = np.asarray(inputs["ln1_b"], f32)
    w_qkv = np.asarray(inputs["w_qkv"], f32)
    b_qkv = np.asarray(inputs["b_qkv"], f32)
    w_dense = np.asarray(inputs["w_dense"], f32)
    b_dense = np.asarray(inputs["b_dense"], f32)
    ln2_g = np.asarray(inputs["ln2_g"], f32)
    ln2_b = np.asarray(inputs["ln2_b"], f32)
    w1 = np.asarray(inputs["w1"], f32)
    b1 = np.asarray(inputs["b1"], f32)
    w2 = np.asarray(inputs["w2"], f32)
    b2 = np.asarray(inputs["b2"], f32)

    H, NH, HD, FF = c.H, c.NH, c.HD, c.FF
    cols = np.concatenate([np.arange(h * 3 * HD, h * 3 * HD + HD)
                           for h in range(NH)])
    wg = ln1_g[:, None] * w_qkv
    wq_f, wk_f, wv_f = wg[:, cols], wg[:, cols + HD], wg[:, cols + 2 * HD]
    bfull = ln1_b @ w_qkv + b_qkv
    bq_f, bk_f, bv_f = bfull[cols], bfull[cols + HD], bfull[cols + 2 * HD]
    bd_f = bv_f @ w_dense + b_dense          # v-bias folded through attention
    w1_f = ln2_g[:, None] * w1
    b1_f = ln2_b @ w1 + b1

    inv = 1.0 / (10000.0 ** (np.arange(0, HD, 2, dtype=f32) / HD))
    pos = np.arange(c.S, dtype=f32)
    frq = np.einsum('i,j->ij', pos, inv)
    emb = np.concatenate([frq, frq], axis=-1)
    cos_full = np.cos(emb).T.astype(f32)
    sin_full = np.sin(emb).T.astype(f32)
    sins_full = sin_full.copy()
    sins_full[:HD // 2] *= -1.0

    bf = lambda a: np.ascontiguousarray(a.astype(BF))
    KH = H // 128
    # [H, H] head-major -> [128, NH*KH*128]: col block h*KH*128+(kk*128+f)
    # = w[kk*128+p, h*128+f] (contiguous per-head panel for one-descriptor DMA)
    relay = lambda w: (w.reshape(KH, 128, NH, 128).transpose(1, 2, 0, 3)
                       .reshape(128, NH * KH * 128))
    wqT, wkT = relay(wq_f), relay(wk_f)
    in_maps = []
    for i in range(c.W):
        b, g = i // c.NG, i % c.NG
        t0 = g * c.T
        qpos = np.arange(t0, t0 + c.T)
        kpos = np.arange(c.S)
        mask = (kpos[:, None] <= qpos[None, :]).astype(BF)
        in_maps.append({
            "x": np.ascontiguousarray(hs[b, t0:t0 + c.T, :]),
            "wq": bf(wqT), "wk": bf(wkT), "wv": bf(wv_f),
            "wd": bf(w_dense), "w1": bf(w1_f), "w2": bf(w2),
            "bq": bf(bq_f.reshape(1, H)), "bk": bf(bk_f.reshape(1, H)),
            "bd": bf(bd_f.reshape(1, H)),
            "b1": np.ascontiguousarray(b1_f.reshape(FF // 128, 128).T),
            "b2": bf(b2.reshape(1, H)),
            "ones_r": np.ones((1, c.T), BF),
            "ones_c": np.ones((128, 1), BF),
            "cosT": np.ascontiguousarray(cos_full[:, t0:t0 + c.T]),
            "sinsT": np.ascontiguousarray(sins_full[:, t0:t0 + c.T]),
            "maskT": np.ascontiguousarray(mask),
        })
    return in_maps


def assemble_output(c: Cfg, results):
    out = np.empty((c.B, c.S, c.H), np.float32)
    for i in range(c.W):
        b, g = i // c.NG, i % c.NG
        out[b, g * c.T:(g + 1) * c.T, :] = results[i]["out"]
    return out


def run(nc, c: Cfg, inputs, trace=False, **kw):
    in_maps = prepare_in_maps(c, inputs)
    last = None
    for attempt in range(3):
        try:
            res = bass_utils.run_bass_kernel_spmd(
                nc, in_maps, core_ids=list(range(c.W)), trace=trace, **kw)
            return assemble_output(c, res.results), res
        except Exception as e:
            last = e
            print(f"run attempt {attempt} failed: {type(e).__name__}: {e}",
                  file=sys.stderr)
    raise last


# ======================= harness entry point =======================

_CACHE = {}


def kernel(**inputs):
    """Full-input entry: shard, compile (cached), run on 8 cores, gather."""
    c = Cfg()
    if "nc" not in _CACHE:
        _CACHE["nc"] = build(c)
    out, _ = run(_CACHE["nc"], c, inputs, trace=False)
    return out

